# revision 50
# baseline (speedup 1.0000x reference)
"""CLIP contrastive loss on 8 Trainium2 NeuronCores (Bass/Tile), fp8 DoubleRow.

Strategy (data-parallel over image rows, hint's local_loss path):
  - Core c holds image rows [c*1024, (c+1)*1024) and the FULL text matrix.
  - Text rows are rolled by c*1024 on the host so every core's diagonal
    block sits at local cols [0, 1024) (the compiled program is
    core-independent).
  - Features are scaled by 16 on the host and quantized to fp8 e4m3; the
    PE runs DoubleRow matmuls (2 fp8 k-planes per cell, K=256 per MM,
    out width 512 = the s3d3 ISA max) at 2x bf16 throughput.
  - Loop: for each 2048-col group (4 of them), for each 128-row m-tile
    (8): 2 kc x 4 DoubleRow MMs -> [128, 2048] PSUM (4 banks, 2-deep
    ping-pong), then ONE ACT exp over the whole 4-bank span (bf16 out,
    accum_out = row-sum partials) and a DVE tensor_add into the group's
    column accumulator.  ACT is the steady-state gate at ~2.04us/slot
    (exp streams 1 elem/cycle/lane at 1.2GHz + READ_ACCUMULATOR).
  - Startup care (measured on perfetto traces): each dma_start costs
    ~620ns of SERIAL descriptor generation (DIRECT2D) on the sync
    sequencer, and SDMA engines split bandwidth evenly over all queued
    transfers.  So: `at` is staged [128, MT, KC, 2, 128] (every DMA
    contiguous >=512B/partition), the critical prefix (at mt0/mt1 + g0's
    four half tiles) is dispatched first, and the 3MB of g1-3 loads are
    emitted *behind tiny DVE reads of their target tiles* so descriptor
    generation is deferred until g0 is rolling (release_bulk).  The PE's
    cold-start ramp (~430ns/MM for the first ~3us) is burned on 6 dummy
    matmuls over a memset tile during the DMA wait.  g0/mt0 runs as two
    [128,1024] EXPs on independent PSUM half-tiles: first EXP ~12.6us.
  - Tail care: g3/mt7 also splits in halves; its exps ship RAW (host
    adds them into g3's colsums) so no ADD sits between the last EXP and
    the last DMA; colacc(mt0-6) goes out during mt7.  Remaining tail =
    dispatch + 0.25MB stream + ~1.9us HBM write receipt + ~2.4us barrier.
  - Diagonals all live in group 0 (cols mt*128..mt*128+128): their bf16
    exp values are staged via DVE copies and shipped once; the host
    recovers diag = log(e) + shift (error ~0.4% pointwise, averages out).
  - Host: partition-reduce column accumulators, combine row/col exp-sums
    and diagonals in float64: lse = shift + log(sum); mean over both
    directions.

Fixed-shift logsumexp is numerically safe: logits are bounded by +-scale
and shift = scale/2 keeps every term that matters in normal f32 range.

Measured: ~87.3-88.8us HW exec (full clock; the device sporadically runs
a 1.2x-slower DVFS state) vs 94.4us previous best, rel err 5.2e-4.
Steady state is a gapless ACT stream: 32 EXP slots x (1.86us exp + 0.18us
READ_ACCUMULATOR) = 66.4us busy -- the scalar engine's 1 elem/cycle/lane
hardware floor.  Rowsums must stay on ACT accum_out: DVE pays a
pipe-drain (~op-266ns) after every op, so a second DVE op per slot
(reduce or fused add+accum) exceeds the 2.04us cadence; tensor_reduce is
1x-rate; GPSIMD has no PSUM port and no stock free-dim reduce.
"""

from contextlib import ExitStack

import numpy as np
import ml_dtypes

import concourse.bass as bass
from concourse import bacc
import concourse.tile as tile
from concourse import mybir
from concourse.bass import ts
from concourse.bass_utils import run_bass_kernel_spmd

N = 8192
D = 512
NC = 8
M_LOC = N // NC          # 1024 image rows per core
MT = M_LOC // 128        # 8 m-tiles of 128 rows
NG = 4                   # column groups
GW = N // NG             # 2048 cols per group
HW = GW // 2             # half-group width (split slots)
HW2 = GW // 4            # quarter width (first-MM DMA chunk)
KC = 2                   # DoubleRow K-chunks (256 each)
PRE = 16.0               # host-side fp8 pre-scale per operand
RS = MT + 1              # rowr slots per group (mt0/mt7 splits add one)

F32 = mybir.dt.float32
BF16 = mybir.dt.bfloat16
FP8 = mybir.dt.float8e4

# moving-free per matmul: out width = MM_W, moving fp8 elements = 2*MM_W
MM_W = 512

_CACHE = {}
LAST_RESULTS = None


def _build(scale: float, shift: float, mm_w: int):
    n_mm = GW // mm_w            # matmuls per (kc, group-span)
    act_scale = scale / (PRE * PRE)
    nc = bacc.Bacc("TRN2", debug=False)

    at_d = nc.dram_tensor("at_in", [128, MT, KC, 2, 128], FP8, kind="ExternalInput").ap()
    bt_d = nc.dram_tensor("bt_in", [NG, KC, 128, 2, GW], FP8, kind="ExternalInput").ap()

    rowr_d = nc.dram_tensor("rowr_out", [128, NG, RS], F32, kind="ExternalOutput").ap()
    # slot NG holds g3/mt7's raw exp values (no final ADD on the critical
    # tail); the host adds it into g3's column sums.
    colsum_d = nc.dram_tensor("colsum_out", [NG + 1, 128, GW], BF16, kind="ExternalOutput").ap()
    # bf16 exp values of the diagonal blocks; host recovers the diagonal
    # logits as log(e) + shift (bf16 log error ~0.4% averages out over N).
    ediag_d = nc.dram_tensor("ediag_out", [128, MT, 128], BF16, kind="ExternalOutput").ap()

    with ExitStack() as ctx:
        tc = ctx.enter_context(tile.TileContext(nc))
        singles = ctx.enter_context(tc.tile_pool(name="singles", bufs=1))
        btp = ctx.enter_context(tc.tile_pool(name="btp", bufs=NG * KC))
        expp = ctx.enter_context(tc.tile_pool(name="expp", bufs=3))
        cap = ctx.enter_context(tc.tile_pool(name="cap", bufs=2))
        psum = ctx.enter_context(tc.tile_pool(name="psum", bufs=2, space="PSUM"))

        # Warm-up chain first in program order: nothing here depends on DMA,
        # so the ACT table set loads during NEFF bring-up instead of gating
        # the first real EXP.
        bias_t = singles.tile([128, 1], F32)
        nc.vector.memset(bias_t, -shift)
        warm_t = singles.tile([128, 1], F32)
        nc.scalar.activation(
            warm_t, bias_t, mybir.ActivationFunctionType.Exp, bias=0.0, scale=1.0
        )
        # PE ramp-up: cold matmuls run 2-3x slow for the first ~2-3us.  Burn
        # the ramp on dummy MMs over a memset tile while the critical DMA is
        # still streaming, so the first real MMs run at the warm 216ns rate.
        wsrc = singles.tile([128, 2, MM_W], FP8)
        nc.gpsimd.memset(wsrc, 0.0)

        at_t = singles.tile([128, MT, KC, 2, 128], FP8)
        # g0 uses two independent half tiles per kc so the first EXP is
        # gated only by the left halves; g1-3 use one whole tile per kc
        # (fewer serial DIRECT2D dispatches) exposed as [L, R] views.
        bt_tiles = [
            [
                [
                    btp.tile([128, 2, HW], FP8, name=f"bt0_{kc}_{h}", tag="bth")
                    for h in range(2)
                ]
                for kc in range(KC)
            ]
        ]
        bt_whole = {}
        for g in range(1, NG):
            row = []
            for kc in range(KC):
                w = btp.tile([128, 2, GW], FP8, name=f"bt{g}_{kc}", tag="bt")
                bt_whole[(g, kc)] = w
                row.append([w[:, :, 0:HW], w[:, :, HW:GW]])
            bt_tiles.append(row)
        # Critical-prefix DMA order.  Each dma_start costs ~620ns of SERIAL
        # HWDGE descriptor generation (DIRECT2D) on the sync sequencer, so
        # keep the count low and dispatch strictly in consumption order:
        # the first EXP (g0/mt0 left half) needs at mt0 + bt0x[:, :, 0:HW].
        nc.sync.dma_start(at_t[:, 0], at_d[:, 0])
        nc.sync.dma_start(bt_tiles[0][0][0], bt_d[0, 0][:, :, 0:HW])
        nc.sync.dma_start(bt_tiles[0][1][0], bt_d[0, 1][:, :, 0:HW])
        nc.sync.dma_start(at_t[:, 1], at_d[:, 1])

        # Pacing gadget: the SDMA engines split bandwidth evenly over all
        # queued transfers, so the 3MB of g1-3 loads would starve the
        # critical g0 stream if dispatched now.  Instead each bulk
        # dma_start is emitted behind a tiny DVE read of its target tile;
        # the DVE reaches that read only after a chosen EXP, so descriptor
        # generation (and thus the transfer) is deferred until the g0
        # pipeline is rolling.
        paceA = singles.tile([128, NG], F32)
        paceB = singles.tile([128, KC, NG], FP8)

        def release_bulk(g, mt):
            nc.vector.tensor_copy(paceA[:, g : g + 1], rowr[:, 0, mt : mt + 1])
            for kc in range(KC):
                nc.vector.tensor_copy(
                    paceB[:, kc, g : g + 1], bt_whole[(g, kc)][:, 0, 0:1]
                )
                nc.sync.dma_start(bt_whole[(g, kc)], bt_d[g, kc])

        rowr = singles.tile([128, NG, RS], F32)
        # diag-block exp values staged on-chip (DVE copies), one DMA at g0
        # end: per-slot DMA readers of e-tiles can get stuck behind bulk
        # loads on a single SDMA engine and stall the e-tile ring.
        dsave = singles.tile([128, MT, 128], BF16)

        def mm_span(s_ps, g, mt, w0, w1, psum_off):
            for kc in range(KC):
                lhsT = at_t[:, mt, kc]                         # [128, 2, 128]
                for w in range(w0, w1):
                    h, wh = divmod(w, HW // mm_w)
                    nc.tensor.matmul(
                        s_ps[:, w * mm_w - psum_off : (w + 1) * mm_w - psum_off],
                        lhsT,
                        bt_tiles[g][kc][h][:, :, ts(wh, mm_w)],
                        start=(kc == 0),
                        stop=(kc == KC - 1),
                        perf_mode=mybir.MatmulPerfMode.DoubleRow,
                    )

        def do_exp(s_ps, lo, hi, g, slot, tag):
            e_t = expp.tile([128, hi - lo], BF16, name=f"e{tag}", tag="exp")
            nc.scalar.activation(
                e_t,
                s_ps[:, lo:hi],
                mybir.ActivationFunctionType.Exp,
                bias=bias_t,
                scale=act_scale,
                accum_out=rowr[:, g, slot : slot + 1],
            )
            return e_t

        warm_ps = psum.tile([128, MM_W], F32, name="warm_ps", tag="spsum")
        for _ in range(6):
            nc.tensor.matmul(
                warm_ps,
                wsrc[:, :, 0:128],
                wsrc,
                start=True,
                stop=True,
                perf_mode=mybir.MatmulPerfMode.DoubleRow,
            )
        # Defer the right halves + at[2:] until the warm chain retires so
        # the critical left-half stream gets the full HBM share; the B-half
        # MMs need R only from ~12.5us.
        paceW = singles.tile([128, 4], F32)
        nc.vector.tensor_copy(paceW[:, 0:1], warm_ps[:, 0:1])
        nc.vector.tensor_copy(paceW[:, 1:2].bitcast(FP8)[:, 0:1], bt_tiles[0][0][1][:, 0, 0:1])
        nc.sync.dma_start(bt_tiles[0][0][1], bt_d[0, 0][:, :, HW:GW])
        nc.vector.tensor_copy(paceW[:, 2:3].bitcast(FP8)[:, 0:1], bt_tiles[0][1][1][:, 0, 0:1])
        nc.sync.dma_start(bt_tiles[0][1][1], bt_d[0, 1][:, :, HW:GW])
        nc.vector.tensor_copy(paceW[:, 3:4].bitcast(FP8)[:, 0:1], at_t[:, 2, 0, 0, 0:1])
        nc.sync.dma_start(at_t[:, 2:MT], at_d[:, 2:MT])

        for g in range(NG):
            last_g = g == NG - 1
            if last_g:
                # split column accumulator so the left colsum DMA can fire
                # right after the left half of the last ADD (tile deps are
                # whole-tile, not subtile).
                colaccL = cap.tile([128, HW], BF16, name="caccL", tag="cacc")
                colaccR = cap.tile([128, HW], BF16, name="caccR", tag="cacc")
            else:
                colacc = cap.tile([128, GW], BF16, name=f"cacc{g}", tag="cacc")
            for mt in range(MT):
                first_split = g == 0 and mt == 0
                last_split = last_g and mt == MT - 1
                if first_split:
                    # split first slot: the A-half EXP fires as soon as the
                    # left-half bt data + 4 MMs are done.  B's MMs are
                    # emitted before eA so the PE-side semaphore for eB
                    # lands right at B's last producer.
                    sA = psum.tile([128, HW], F32, name="sA0", tag="spsum")
                    sB = psum.tile([128, HW], F32, name="sB0", tag="spsum")
                    mm_span(sA, g, mt, 0, n_mm // 2, 0)
                    mm_span(sB, g, mt, n_mm // 2, n_mm, HW)
                    eA = do_exp(sA, 0, HW, g, 0, "0A")
                    eB = do_exp(sB, 0, HW, g, MT, "0B")
                    nc.vector.tensor_copy(colacc[:, 0:HW], eA)
                    nc.vector.tensor_copy(colacc[:, HW:GW], eB)
                    nc.vector.tensor_copy(dsave[:, 0], eA[:, 0:128])
                    release_bulk(1, 0)
                elif last_split:
                    # two independent [128, HW] PSUM tiles so the left half
                    # EXP / DMA doesn't wait on the right half's MMs.
                    sA = psum.tile([128, HW], F32, name=f"sA{g}", tag="spsum")
                    sB = psum.tile([128, HW], F32, name=f"sB{g}", tag="spsum")
                    mm_span(sA, g, mt, 0, n_mm // 2, 0)
                    # mt7's exps ship raw (summed on host): no ADD or
                    # wide DMA behind the last EXP.
                    eL = do_exp(sA, 0, HW, g, MT - 1, "7L")
                    mm_span(sB, g, mt, n_mm // 2, n_mm, HW)
                    nc.sync.dma_start(colsum_d[NG][:, 0:HW], eL)
                    eR = do_exp(sB, 0, HW, g, MT, "7R")
                    nc.sync.dma_start(colsum_d[NG][:, HW:GW], eR)
                    nc.sync.dma_start(rowr_d, rowr)
                else:
                    s_ps = psum.tile([128, GW], F32, name=f"s{g}_{mt}", tag="spsum")
                    mm_span(s_ps, g, mt, 0, n_mm, 0)
                    e_t = do_exp(s_ps, 0, GW, g, mt, str((g, mt)))
                    if last_g:
                        if mt == 0:
                            nc.vector.tensor_copy(colaccL, e_t[:, 0:HW])
                            nc.vector.tensor_copy(colaccR, e_t[:, HW:GW])
                        else:
                            nc.vector.tensor_add(colaccL, colaccL, e_t[:, 0:HW])
                            nc.vector.tensor_add(colaccR, colaccR, e_t[:, HW:GW])
                            if mt == MT - 2:
                                # colacc (mt0-6) done: overlap its DMA with
                                # the mt7 tail.
                                nc.sync.dma_start(colsum_d[g][:, 0:HW], colaccL)
                                nc.sync.dma_start(colsum_d[g][:, HW:GW], colaccR)
                    elif mt == 0:
                        nc.vector.tensor_copy(colacc, e_t)
                    else:
                        nc.vector.tensor_add(colacc, colacc, e_t)
                    if g == 0:
                        # diag block for mt sits at local cols
                        # [mt*128, mt*128+128); stage its exp values.
                        nc.vector.tensor_copy(dsave[:, mt], e_t[:, ts(mt, 128)])
                        if mt == 2:
                            release_bulk(2, 2)
                        elif mt == 4:
                            release_bulk(3, 4)
            if not last_g:
                nc.sync.dma_start(colsum_d[g], colacc)
            if g == 0:
                nc.sync.dma_start(ediag_d, dsave)

    nc.compile()
    return nc


def _prep_inputs(img, txt, scale):
    fp8 = ml_dtypes.float8_e4m3fn
    in_maps = []
    for c in range(NC):
        A = (PRE * img[c * M_LOC : (c + 1) * M_LOC]).astype(fp8)   # [1024, 512]
        # at[p, mt, kc, ko, j] = A[mt*128+j, kc*256+ko*128+p]
        at = np.ascontiguousarray(
            A.T.reshape(KC, 2, 128, MT, 128).transpose(2, 3, 0, 1, 4)
        )                                                          # [128, MT, KC, 2, 128]
        tr = np.roll(txt, -c * M_LOC, axis=0)                      # local col j -> global (j + c*1024) % N
        B = (PRE * tr).astype(fp8)                                 # [8192, 512]
        bt = np.ascontiguousarray(
            B.T.reshape(KC, 2, 128, NG, GW).transpose(3, 0, 2, 1, 4)
        )                                                          # [NG, KC, 128, 2, GW]
        in_maps.append({"at_in": at, "bt_in": bt})
    return in_maps


def kernel(image_features, text_features, logit_scale):
    global LAST_RESULTS
    img = np.ascontiguousarray(np.asarray(image_features, dtype=np.float32))
    txt = np.ascontiguousarray(np.asarray(text_features, dtype=np.float32))
    scale = float(np.asarray(logit_scale))
    shift = 0.5 * scale

    key = (scale, MM_W)
    if key not in _CACHE:
        _CACHE[key] = _build(scale, shift, MM_W)
    nc = _CACHE[key]

    in_maps = _prep_inputs(img, txt, scale)
    res = None
    last_err = None
    for _attempt in range(3):
        try:
            res = run_bass_kernel_spmd(nc, in_maps, core_ids=list(range(NC)))
            break
        except Exception as e:  # transient NRT/device hiccups: retry
            last_err = e
    if res is None:
        raise last_err
    LAST_RESULTS = res

    colsum_tot = np.zeros(N, dtype=np.float64)
    lse_rows = []
    diags = []
    for c, r in enumerate(res.results):
        rr = r["rowr_out"].astype(np.float64)                       # [128, NG, RS]
        # rowsum partials per (p, mt): slot mt for each group, plus the
        # extra slot RS-1 holding g0/mt0's right half and g3/mt7's right
        # half respectively.
        per_mt = rr[:, :, :MT].sum(axis=1)                          # [128, MT]
        per_mt[:, 0] += rr[:, 0, MT]                                # g0 mt0 B-half
        per_mt[:, MT - 1] += rr[:, NG - 1, MT]                      # g3 mt7 R-half
        lse_rows.append(shift + np.log(per_mt.T.reshape(-1)))       # row = mt*128 + p
        ed = r["ediag_out"].astype(np.float64)                      # [128, MT, 128]
        e_diag = ed[np.arange(128), :, np.arange(128)]              # [128, MT]
        diags.append((np.log(e_diag) + shift).T.reshape(-1))        # row = mt*128 + p
        cs = r["colsum_out"].astype(np.float64).sum(axis=1)         # [NG+1, GW]
        cs[NG - 1] += cs[NG]
        colsum_tot += np.roll(cs[:NG].reshape(-1), c * M_LOC)
    lse_row = np.concatenate(lse_rows)
    diag = np.concatenate(diags)
    lse_col = shift + np.log(colsum_tot)

    loss = 0.5 * (np.mean(lse_row - diag) + np.mean(lse_col - diag))
    return np.float32(loss)


# revision 51
# speedup vs baseline: 1.0087x; 1.0087x over previous
"""CLIP contrastive loss on 8 Trainium2 NeuronCores (Bass/Tile), fp8 DoubleRow.

Strategy (data-parallel over image rows, hint's local_loss path):
  - Core c holds image rows [c*1024, (c+1)*1024) and the FULL text matrix.
  - Text rows are rolled by c*1024 on the host so every core's diagonal
    block sits at local cols [0, 1024) (the compiled program is
    core-independent).
  - Features are scaled by 16 on the host and quantized to fp8 e4m3; the
    PE runs DoubleRow matmuls (2 fp8 k-planes per cell, K=256 per MM,
    out width 512 = the s3d3 ISA max) at 2x bf16 throughput.
  - Loop: for each 2048-col group (4 of them), for each 128-row m-tile
    (8): 2 kc x 4 DoubleRow MMs -> [128, 2048] PSUM (4 banks, 2-deep
    ping-pong), then ONE ACT exp over the whole 4-bank span (bf16 out,
    accum_out = row-sum partials) and a DVE tensor_add into the group's
    column accumulator.  ACT is the steady-state gate at ~2.04us/slot
    (exp streams 1 elem/cycle/lane at 1.2GHz + READ_ACCUMULATOR).
  - Startup care (measured on perfetto traces): each dma_start costs
    ~620ns of SERIAL descriptor generation (DIRECT2D) on the sync
    sequencer, and SDMA engines split bandwidth evenly over all queued
    transfers.  So: `at` is staged [128, MT, KC, 2, 128] (every DMA
    contiguous >=512B/partition), the critical prefix (at mt0/mt1 + g0's
    four half tiles) is dispatched first, and the 3MB of g1-3 loads are
    emitted *behind tiny DVE reads of their target tiles* so descriptor
    generation is deferred until g0 is rolling (release_bulk).  The PE's
    cold-start ramp (~430ns/MM for the first ~3us) is burned on 6 dummy
    matmuls over a memset tile during the DMA wait.  g0/mt0 runs as two
    [128,1024] EXPs on independent PSUM half-tiles: first EXP ~12.6us.
  - Tail care: g3/mt7 also splits in halves; its exps ship RAW (host
    adds them into g3's colsums) so no ADD sits between the last EXP and
    the last DMA; colacc(mt0-6) goes out during mt7.  Remaining tail =
    dispatch + 0.25MB stream + ~1.9us HBM write receipt + ~2.4us barrier.
  - Diagonals all live in group 0 (cols mt*128..mt*128+128): their bf16
    exp values are staged via DVE copies and shipped once; the host
    recovers diag = log(e) + shift (error ~0.4% pointwise, averages out).
  - Host: partition-reduce column accumulators, combine row/col exp-sums
    and diagonals in float64: lse = shift + log(sum); mean over both
    directions.

Fixed-shift logsumexp is numerically safe: logits are bounded by +-scale
and shift = scale/2 keeps every term that matters in normal f32 range.

Measured: ~87.3-88.8us HW exec (full clock; the device sporadically runs
a 1.2x-slower DVFS state) vs 94.4us previous best, rel err 5.2e-4.
Steady state is a gapless ACT stream: 32 EXP slots x (1.86us exp + 0.18us
READ_ACCUMULATOR) = 66.4us busy -- the scalar engine's 1 elem/cycle/lane
hardware floor.  Rowsums must stay on ACT accum_out: DVE pays a
pipe-drain (~op-266ns) after every op, so a second DVE op per slot
(reduce or fused add+accum) exceeds the 2.04us cadence; tensor_reduce is
1x-rate; GPSIMD has no PSUM port and no stock free-dim reduce.
"""

from contextlib import ExitStack

import numpy as np
import ml_dtypes

import concourse.bass as bass
from concourse import bacc
import concourse.tile as tile
from concourse import mybir
from concourse.bass import ts
from concourse.bass_utils import run_bass_kernel_spmd

N = 8192
D = 512
NC = 8
M_LOC = N // NC          # 1024 image rows per core
MT = M_LOC // 128        # 8 m-tiles of 128 rows
NG = 4                   # column groups
GW = N // NG             # 2048 cols per group
HW = GW // 2             # half-group width (split slots)
HW2 = GW // 4            # quarter width (first-MM DMA chunk)
KC = 2                   # DoubleRow K-chunks (256 each)
PRE = 16.0               # host-side fp8 pre-scale per operand
RS = MT + 1              # rowr slots per group (mt0/mt7 splits add one)

F32 = mybir.dt.float32
BF16 = mybir.dt.bfloat16
FP8 = mybir.dt.float8e4

# moving-free per matmul: out width = MM_W, moving fp8 elements = 2*MM_W
MM_W = 512

_CACHE = {}
LAST_RESULTS = None


def _build(scale: float, shift: float, mm_w: int):
    n_mm = GW // mm_w            # matmuls per (kc, group-span)
    act_scale = scale / (PRE * PRE)
    nc = bacc.Bacc("TRN2", debug=False)

    at_d = nc.dram_tensor("at_in", [128, MT, KC, 2, 128], FP8, kind="ExternalInput").ap()
    bt_d = nc.dram_tensor("bt_in", [NG, KC, 128, 2, GW], FP8, kind="ExternalInput").ap()

    rowr_d = nc.dram_tensor("rowr_out", [128, NG, RS], F32, kind="ExternalOutput").ap()
    # slot NG holds g3/mt7's raw exp values (no final ADD on the critical
    # tail); the host adds it into g3's column sums.
    colsum_d = nc.dram_tensor("colsum_out", [NG + 1, 128, GW], BF16, kind="ExternalOutput").ap()
    # bf16 exp values of the diagonal blocks; host recovers the diagonal
    # logits as log(e) + shift (bf16 log error ~0.4% averages out over N).
    ediag_d = nc.dram_tensor("ediag_out", [128, MT, 128], BF16, kind="ExternalOutput").ap()

    with ExitStack() as ctx:
        tc = ctx.enter_context(tile.TileContext(nc))
        singles = ctx.enter_context(tc.tile_pool(name="singles", bufs=1))
        btp = ctx.enter_context(tc.tile_pool(name="btp", bufs=NG * KC))
        expp = ctx.enter_context(tc.tile_pool(name="expp", bufs=3))
        cap = ctx.enter_context(tc.tile_pool(name="cap", bufs=2))
        psum = ctx.enter_context(tc.tile_pool(name="psum", bufs=2, space="PSUM"))

        # Warm-up chain first in program order: nothing here depends on DMA,
        # so the ACT table set loads during NEFF bring-up instead of gating
        # the first real EXP.
        bias_t = singles.tile([128, 1], F32)
        nc.vector.memset(bias_t, -shift)
        warm_t = singles.tile([128, 1], F32)
        nc.scalar.activation(
            warm_t, bias_t, mybir.ActivationFunctionType.Exp, bias=0.0, scale=1.0
        )
        # PE ramp-up: cold matmuls run 2-3x slow for the first ~2-3us.  Burn
        # the ramp on dummy MMs over a memset tile while the critical DMA is
        # still streaming, so the first real MMs run at the warm 216ns rate.
        wsrc = singles.tile([128, 2, MM_W], FP8)
        nc.gpsimd.memset(wsrc, 0.0)

        at_t = singles.tile([128, MT, KC, 2, 128], FP8)
        # g0 uses two independent half tiles per kc so the first EXP is
        # gated only by the left halves; g1-3 use one whole tile per kc
        # (fewer serial DIRECT2D dispatches) exposed as [L, R] views.
        bt_tiles = [
            [
                [
                    btp.tile([128, 2, HW], FP8, name=f"bt0_{kc}_{h}", tag="bth")
                    for h in range(2)
                ]
                for kc in range(KC)
            ]
        ]
        bt_whole = {}
        for g in range(1, NG):
            row = []
            for kc in range(KC):
                w = btp.tile([128, 2, GW], FP8, name=f"bt{g}_{kc}", tag="bt")
                bt_whole[(g, kc)] = w
                row.append([w[:, :, 0:HW], w[:, :, HW:GW]])
            bt_tiles.append(row)
        # Critical-prefix DMA order.  Each dma_start costs ~620ns of SERIAL
        # HWDGE descriptor generation (DIRECT2D) on the sync sequencer, so
        # keep the count low and dispatch strictly in consumption order:
        # the first EXP (g0/mt0 left half) needs at mt0 + bt0x[:, :, 0:HW].
        nc.sync.dma_start(at_t[:, 0], at_d[:, 0])
        nc.sync.dma_start(bt_tiles[0][0][0], bt_d[0, 0][:, :, 0:HW])
        nc.sync.dma_start(bt_tiles[0][1][0], bt_d[0, 1][:, :, 0:HW])
        nc.sync.dma_start(at_t[:, 1], at_d[:, 1])
        nc.sync.dma_start(bt_tiles[0][0][1], bt_d[0, 0][:, :, HW:GW])
        nc.sync.dma_start(bt_tiles[0][1][1], bt_d[0, 1][:, :, HW:GW])
        nc.sync.dma_start(at_t[:, 2:MT], at_d[:, 2:MT])

        # Pacing gadget: the SDMA engines split bandwidth evenly over all
        # queued transfers, so the 3MB of g1-3 loads would starve the
        # critical g0 stream if dispatched now.  Instead each bulk
        # dma_start is emitted behind a tiny DVE read of its target tile;
        # the DVE reaches that read only after a chosen EXP, so descriptor
        # generation (and thus the transfer) is deferred until the g0
        # pipeline is rolling.
        paceA = singles.tile([128, NG], F32)
        paceB = singles.tile([128, KC, NG], FP8)

        def release_bulk(g, mt):
            nc.vector.tensor_copy(paceA[:, g : g + 1], rowr[:, 0, mt : mt + 1])
            for kc in range(KC):
                nc.vector.tensor_copy(
                    paceB[:, kc, g : g + 1], bt_whole[(g, kc)][:, 0, 0:1]
                )
                nc.sync.dma_start(bt_whole[(g, kc)], bt_d[g, kc])

        rowr = singles.tile([128, NG, RS], F32)
        # diag-block exp values staged on-chip (DVE copies), one DMA at g0
        # end: per-slot DMA readers of e-tiles can get stuck behind bulk
        # loads on a single SDMA engine and stall the e-tile ring.
        dsave = singles.tile([128, MT, 128], BF16)

        def mm_span(s_ps, g, mt, w0, w1, psum_off):
            for kc in range(KC):
                lhsT = at_t[:, mt, kc]                         # [128, 2, 128]
                for w in range(w0, w1):
                    h, wh = divmod(w, HW // mm_w)
                    nc.tensor.matmul(
                        s_ps[:, w * mm_w - psum_off : (w + 1) * mm_w - psum_off],
                        lhsT,
                        bt_tiles[g][kc][h][:, :, ts(wh, mm_w)],
                        start=(kc == 0),
                        stop=(kc == KC - 1),
                        perf_mode=mybir.MatmulPerfMode.DoubleRow,
                    )

        def do_exp(s_ps, lo, hi, g, slot, tag):
            e_t = expp.tile([128, hi - lo], BF16, name=f"e{tag}", tag="exp")
            nc.scalar.activation(
                e_t,
                s_ps[:, lo:hi],
                mybir.ActivationFunctionType.Exp,
                bias=bias_t,
                scale=act_scale,
                accum_out=rowr[:, g, slot : slot + 1],
            )
            return e_t

        warm_ps = psum.tile([128, MM_W], F32, name="warm_ps", tag="spsum")
        for _ in range(6):
            nc.tensor.matmul(
                warm_ps,
                wsrc[:, :, 0:128],
                wsrc,
                start=True,
                stop=True,
                perf_mode=mybir.MatmulPerfMode.DoubleRow,
            )

        for g in range(NG):
            last_g = g == NG - 1
            if last_g:
                # split column accumulator so the left colsum DMA can fire
                # right after the left half of the last ADD (tile deps are
                # whole-tile, not subtile).
                colaccL = cap.tile([128, HW], BF16, name="caccL", tag="cacc")
                colaccR = cap.tile([128, HW], BF16, name="caccR", tag="cacc")
            else:
                colacc = cap.tile([128, GW], BF16, name=f"cacc{g}", tag="cacc")
            for mt in range(MT):
                first_split = g == 0 and mt == 0
                last_split = last_g and mt == MT - 1
                if first_split:
                    # split first slot: the A-half EXP fires as soon as the
                    # left-half bt data + 4 MMs are done.  B's MMs are
                    # emitted before eA so the PE-side semaphore for eB
                    # lands right at B's last producer.
                    sA = psum.tile([128, HW], F32, name="sA0", tag="spsum")
                    sB = psum.tile([128, HW], F32, name="sB0", tag="spsum")
                    mm_span(sA, g, mt, 0, n_mm // 2, 0)
                    mm_span(sB, g, mt, n_mm // 2, n_mm, HW)
                    eA = do_exp(sA, 0, HW, g, 0, "0A")
                    eB = do_exp(sB, 0, HW, g, MT, "0B")
                    nc.vector.tensor_copy(colacc[:, 0:HW], eA)
                    nc.vector.tensor_copy(colacc[:, HW:GW], eB)
                    nc.vector.tensor_copy(dsave[:, 0], eA[:, 0:128])
                    release_bulk(1, 0)
                elif last_split:
                    # two independent [128, HW] PSUM tiles so the left half
                    # EXP / DMA doesn't wait on the right half's MMs.
                    sA = psum.tile([128, HW], F32, name=f"sA{g}", tag="spsum")
                    sB = psum.tile([128, HW], F32, name=f"sB{g}", tag="spsum")
                    mm_span(sA, g, mt, 0, n_mm // 2, 0)
                    # mt7's exps ship raw (summed on host): no ADD or
                    # wide DMA behind the last EXP.
                    eL = do_exp(sA, 0, HW, g, MT - 1, "7L")
                    mm_span(sB, g, mt, n_mm // 2, n_mm, HW)
                    nc.sync.dma_start(colsum_d[NG][:, 0:HW], eL)
                    eR = do_exp(sB, 0, HW, g, MT, "7R")
                    nc.sync.dma_start(colsum_d[NG][:, HW:GW], eR)
                    nc.sync.dma_start(rowr_d, rowr)
                else:
                    s_ps = psum.tile([128, GW], F32, name=f"s{g}_{mt}", tag="spsum")
                    mm_span(s_ps, g, mt, 0, n_mm, 0)
                    e_t = do_exp(s_ps, 0, GW, g, mt, str((g, mt)))
                    if last_g:
                        if mt == 0:
                            nc.vector.tensor_copy(colaccL, e_t[:, 0:HW])
                            nc.vector.tensor_copy(colaccR, e_t[:, HW:GW])
                        else:
                            nc.vector.tensor_add(colaccL, colaccL, e_t[:, 0:HW])
                            nc.vector.tensor_add(colaccR, colaccR, e_t[:, HW:GW])
                            if mt == MT - 2:
                                # colacc (mt0-6) done: overlap its DMA with
                                # the mt7 tail.
                                nc.sync.dma_start(colsum_d[g][:, 0:HW], colaccL)
                                nc.sync.dma_start(colsum_d[g][:, HW:GW], colaccR)
                    elif mt == 0:
                        nc.vector.tensor_copy(colacc, e_t)
                    else:
                        nc.vector.tensor_add(colacc, colacc, e_t)
                    if g == 0:
                        # diag block for mt sits at local cols
                        # [mt*128, mt*128+128); stage its exp values.
                        nc.vector.tensor_copy(dsave[:, mt], e_t[:, ts(mt, 128)])
                        if mt == 2:
                            release_bulk(2, 2)
                        elif mt == 4:
                            release_bulk(3, 4)
            if not last_g:
                nc.sync.dma_start(colsum_d[g], colacc)
            if g == 0:
                nc.sync.dma_start(ediag_d, dsave)

    nc.compile()
    return nc


def _prep_inputs(img, txt, scale):
    fp8 = ml_dtypes.float8_e4m3fn
    in_maps = []
    for c in range(NC):
        A = (PRE * img[c * M_LOC : (c + 1) * M_LOC]).astype(fp8)   # [1024, 512]
        # at[p, mt, kc, ko, j] = A[mt*128+j, kc*256+ko*128+p]
        at = np.ascontiguousarray(
            A.T.reshape(KC, 2, 128, MT, 128).transpose(2, 3, 0, 1, 4)
        )                                                          # [128, MT, KC, 2, 128]
        tr = np.roll(txt, -c * M_LOC, axis=0)                      # local col j -> global (j + c*1024) % N
        B = (PRE * tr).astype(fp8)                                 # [8192, 512]
        bt = np.ascontiguousarray(
            B.T.reshape(KC, 2, 128, NG, GW).transpose(3, 0, 2, 1, 4)
        )                                                          # [NG, KC, 128, 2, GW]
        in_maps.append({"at_in": at, "bt_in": bt})
    return in_maps


def kernel(image_features, text_features, logit_scale):
    global LAST_RESULTS
    img = np.ascontiguousarray(np.asarray(image_features, dtype=np.float32))
    txt = np.ascontiguousarray(np.asarray(text_features, dtype=np.float32))
    scale = float(np.asarray(logit_scale))
    shift = 0.5 * scale

    key = (scale, MM_W)
    if key not in _CACHE:
        _CACHE[key] = _build(scale, shift, MM_W)
    nc = _CACHE[key]

    in_maps = _prep_inputs(img, txt, scale)
    res = None
    last_err = None
    for _attempt in range(3):
        try:
            res = run_bass_kernel_spmd(nc, in_maps, core_ids=list(range(NC)))
            break
        except Exception as e:  # transient NRT/device hiccups: retry
            last_err = e
    if res is None:
        raise last_err
    LAST_RESULTS = res

    colsum_tot = np.zeros(N, dtype=np.float64)
    lse_rows = []
    diags = []
    for c, r in enumerate(res.results):
        rr = r["rowr_out"].astype(np.float64)                       # [128, NG, RS]
        # rowsum partials per (p, mt): slot mt for each group, plus the
        # extra slot RS-1 holding g0/mt0's right half and g3/mt7's right
        # half respectively.
        per_mt = rr[:, :, :MT].sum(axis=1)                          # [128, MT]
        per_mt[:, 0] += rr[:, 0, MT]                                # g0 mt0 B-half
        per_mt[:, MT - 1] += rr[:, NG - 1, MT]                      # g3 mt7 R-half
        lse_rows.append(shift + np.log(per_mt.T.reshape(-1)))       # row = mt*128 + p
        ed = r["ediag_out"].astype(np.float64)                      # [128, MT, 128]
        e_diag = ed[np.arange(128), :, np.arange(128)]              # [128, MT]
        diags.append((np.log(e_diag) + shift).T.reshape(-1))        # row = mt*128 + p
        cs = r["colsum_out"].astype(np.float64).sum(axis=1)         # [NG+1, GW]
        cs[NG - 1] += cs[NG]
        colsum_tot += np.roll(cs[:NG].reshape(-1), c * M_LOC)
    lse_row = np.concatenate(lse_rows)
    diag = np.concatenate(diags)
    lse_col = shift + np.log(colsum_tot)

    loss = 0.5 * (np.mean(lse_row - diag) + np.mean(lse_col - diag))
    return np.float32(loss)
